# revision 18
# baseline (speedup 1.0000x reference)
"""Trainium2 Bass kernel for LinearTransformerExpert.

Reference computation (per token n, 16 heads, head_dim 128, prefix len 8):
    q = x @ Wq.T ;  k = prefix @ Wk.T ;  v = prefix @ Wv.T
    scores[n,h,p] = q[n,h,:] . k[n,p,h,:] / sqrt(D)
    attn = softmax_p(scores);  out[n,h,:] = sum_p attn * v
    result = (out @ Wo.T) * sigmoid(x @ Wg.T + bg)

Strategy: data-parallel over the 8192 tokens across 8 NeuronCores (1024
tokens each), no collectives. All matmuls run in bf16 on the PE with the
activations as the stationary operand, so every projection lands in PSUM
token-major ([token, out_channel]) — exactly the layout the attention
(computed on DVE/ACT, hidden under the PE) wants.

v2.1 layout: phases split into half-output-channel passes so each pass
needs only 8MB of weights before its first matmul, and the next pass's
weights stream in under compute.
  A (x2 passes): q half (+gate z half, sigmoid) from x; q/g spilled to
     DRAM bf16 (cheap, overlapped). x tiles stay resident across passes.
  B (x2 passes): k/v half projections + attention on those 8 heads, fused
     per (tile, p); attention output ob written by halves, stays in SBUF.
  C0: PE-transpose ob -> oT (SBUF), overlapping the Wo weight DMA.
  C: final = (oT @ Wo.T) * g, streamed out.
Wq is pre-scaled by 1/sqrt(D) on host.
"""

import math

import numpy as np
import ml_dtypes

import concourse.bass as bass
import concourse.bacc as bacc
import concourse.mybir as mybir
from concourse import tile
from concourse.masks import make_identity

BF16 = mybir.dt.bfloat16
F32 = mybir.dt.float32
NPBF16 = ml_dtypes.bfloat16

# problem shape (hardcoded; kernel.py must be self-contained)
N, P, C, H = 8192, 8, 2048, 16
D = C // H                    # 128
NCORES = 8
NTOK = N // NCORES            # 1024 tokens per core
NT = NTOK // 128              # 8 token-tiles per core
CT = C // 128                 # 16 contraction tiles
OH = 2                        # output-channel halves (1024 each)
CH = C // OH                  # 1024
HPH = H // OH                 # heads per half = 8
SCALE = 1.0 / math.sqrt(D)


def _bcast(ap, n):
    """Append a 0-stride broadcast dim of size n to an AP."""
    return bass.AP(ap.tensor, ap.offset, list(ap.ap) + [[0, n]])


def build_nc(nt=NT):
    """Build the per-core SPMD program for `nt` token-tiles (nt*128 tokens)."""
    ntok = nt * 128
    nc = bacc.Bacc("TRN2", target_bir_lowering=False, debug=False,
                   num_devices=NCORES)

    xT = nc.dram_tensor("xT", [C, ntok], BF16, kind="ExternalInput")
    pT = nc.dram_tensor("pT", [C, P, ntok], BF16, kind="ExternalInput")
    wq = nc.dram_tensor("wq", [C, C], BF16, kind="ExternalInput")
    wk = nc.dram_tensor("wk", [C, C], BF16, kind="ExternalInput")
    wv = nc.dram_tensor("wv", [C, C], BF16, kind="ExternalInput")
    wo = nc.dram_tensor("wo", [C, C], BF16, kind="ExternalInput")
    wg = nc.dram_tensor("wg", [C, C], BF16, kind="ExternalInput")
    bg = nc.dram_tensor("bg", [1, C], BF16, kind="ExternalInput")
    bgr = nc.dram_tensor("bgr", [128, C], BF16, kind="ExternalInput")
    out = nc.dram_tensor("out", [ntok, C], F32, kind="ExternalOutput")

    q_spill = nc.dram_tensor("q_spill", [nt, 128, C], BF16)
    g_spill = nc.dram_tensor("g_spill", [nt, 128, C], BF16)

    def wview(w):  # [C, C] dram -> [128, CT, C] (partition, ktile, outch)
        return w[:].rearrange("(t p) o -> p t o", p=128)

    def load_w(pool, w, tag, o0=0, owid=C):
        """Chunked (per c-tile) weight DMA into an SBUF tile; returns 3D view."""
        sb = pool.tile([128, CT * owid], BF16, tag=tag)
        v = sb[:].rearrange("p (t o) -> p t o", t=CT)
        wv_ = wview(w)
        for t in range(CT):
            nc.sync.dma_start(out=v[:, t, :], in_=wv_[:, t, o0:o0 + owid])
        return v

    def xview(j):  # stationary block for token-tile j: [128, CT, 128]
        return xT[:, j * 128:(j + 1) * 128].rearrange("(t p) n -> p t n", p=128)

    def pview(j, p):
        return pT[:, p, j * 128:(j + 1) * 128].rearrange("(t p) n -> p t n", p=128)

    with tile.TileContext(nc) as tc:
        with tc.tile_pool(name="const", bufs=1) as pconst:
            ident = pconst.tile([128, 128], BF16, tag="ident")
            make_identity(nc, ident[:])

            # ---------- Phase A: q = x@Wq.T (pre-scaled), g = sigmoid(x@Wg.T+bg)
            # Two half-output passes; x tiles resident across both.
            with tc.tile_pool(name="pax", bufs=1) as pax:
                xb_v = []
                with tc.tile_pool(name="paw", bufs=2) as paw, \
                     tc.tile_pool(name="pao", bufs=3) as pao, \
                     tc.tile_pool(name="papq", bufs=2, space="PSUM") as papq, \
                     tc.tile_pool(name="papz", bufs=2, space="PSUM") as papz:
                    bgr_sb = pax.tile([128, C], BF16, tag="bgr")
                    nc.sync.dma_start(out=bgr_sb[:], in_=bgr[:])
                    for hf in range(OH):
                        o0 = hf * CH
                        wq_v = load_w(paw, wq, "wq", o0, CH)
                        wg_v = load_w(paw, wg, "wg", o0, CH)
                        for j in range(nt):
                            if hf == 0:
                                xb = pax.tile([128, CT * 128], BF16,
                                              tag=f"xb{j}")
                                xb_v.append(xb[:].rearrange(
                                    "p (t n) -> p t n", t=CT))
                                nc.sync.dma_start(out=xb_v[j], in_=xview(j))
                            qp = papq.tile([128, CH], F32, tag="qp")
                            zp = papz.tile([128, CH], F32, tag="zp")
                            for t in range(CT):
                                for c2 in range(CH // 512):
                                    sl = slice(c2 * 512, (c2 + 1) * 512)
                                    nc.tensor.matmul(
                                        qp[:, sl], xb_v[j][:, t, :],
                                        wq_v[:, t, sl],
                                        start=(t == 0), stop=(t == CT - 1))
                                    nc.tensor.matmul(
                                        zp[:, sl], xb_v[j][:, t, :],
                                        wg_v[:, t, sl],
                                        start=(t == 0), stop=(t == CT - 1))
                            qb = pao.tile([128, CH], BF16, tag="qb")
                            nc.scalar.copy(qb[:], qp[:])
                            nc.sync.dma_start(
                                out=q_spill[j, :, o0:o0 + CH], in_=qb[:])
                            zb = pao.tile([128, CH], BF16, tag="zb")
                            nc.vector.tensor_add(
                                zb[:], zp[:], bgr_sb[:, o0:o0 + CH])
                            gb = pao.tile([128, CH], BF16, tag="gb")
                            nc.scalar.activation(
                                gb[:], zb[:],
                                mybir.ActivationFunctionType.Sigmoid)
                            nc.sync.dma_start(
                                out=g_spill[j, :, o0:o0 + CH], in_=gb[:])

            # ---------- Phase B: k/v projection + attention, fused ----------
            # Two half-output passes (8 heads each); ob written by halves.
            with tc.tile_pool(name="pbob", bufs=1) as pbob:
                ob = []
                with tc.tile_pool(name="pbw", bufs=2) as pbw, \
                     tc.tile_pool(name="pbpf", bufs=3) as pbpf, \
                     tc.tile_pool(name="pbq", bufs=2) as pbq, \
                     tc.tile_pool(name="pbkv", bufs=2) as pbkv, \
                     tc.tile_pool(name="pbO", bufs=2) as pbO, \
                     tc.tile_pool(name="pbsc", bufs=2) as pbsc, \
                     tc.tile_pool(name="pbsm", bufs=3) as pbsm, \
                     tc.tile_pool(name="pbpsk", bufs=2, space="PSUM") as pbpsk, \
                     tc.tile_pool(name="pbpsv", bufs=2, space="PSUM") as pbpsv:
                    for hf in range(OH):
                        o0 = hf * CH
                        wk_v = load_w(pbw, wk, "wk", o0, CH)
                        wv_v = load_w(pbw, wv, "wv", o0, CH)
                        for j in range(nt):
                            qb = pbq.tile([128, CH], BF16, tag="q")
                            nc.sync.dma_start(out=qb[:],
                                              in_=q_spill[j, :, o0:o0 + CH])
                            if hf == 0:
                                ob_j = pbob.tile([128, C], BF16, tag=f"ob{j}")
                                ob.append(ob_j)
                            O = pbO.tile([128, CH], F32, tag="O")
                            s_den = pbsm.tile([128, HPH], F32, tag="sden")
                            for p in range(P):
                                pf = pbpf.tile([128, CT * 128], BF16, tag="pf")
                                pf_v = pf[:].rearrange("p (t n) -> p t n", t=CT)
                                nc.sync.dma_start(out=pf_v, in_=pview(j, p))
                                kp = pbpsk.tile([128, CH], F32, tag="kp")
                                vp = pbpsv.tile([128, CH], F32, tag="vp")
                                for t in range(CT):
                                    for c2 in range(CH // 512):
                                        sl = slice(c2 * 512, (c2 + 1) * 512)
                                        nc.tensor.matmul(
                                            kp[:, sl], pf_v[:, t, :],
                                            wk_v[:, t, sl],
                                            start=(t == 0), stop=(t == CT - 1))
                                        nc.tensor.matmul(
                                            vp[:, sl], pf_v[:, t, :],
                                            wv_v[:, t, sl],
                                            start=(t == 0), stop=(t == CT - 1))
                                kb = pbkv.tile([128, CH], BF16, tag="kb")
                                vb = pbkv.tile([128, CH], BF16, tag="vb")
                                nc.scalar.copy(kb[:], kp[:])
                                nc.vector.tensor_copy(vb[:], vp[:])
                                # scores for the 8 heads of this half
                                prod = pbsc.tile([128, CH], F32, tag="prod")
                                nc.vector.tensor_mul(prod[:], qb[:], kb[:])
                                sc = pbsm.tile([128, HPH], F32, tag="sc")
                                nc.vector.tensor_reduce(
                                    sc[:],
                                    prod[:].rearrange("p (h d) -> p h d", d=D),
                                    mybir.AxisListType.X, mybir.AluOpType.add)
                                ee = pbsm.tile([128, HPH], F32, tag="ee")
                                nc.scalar.activation(
                                    ee[:], sc[:],
                                    mybir.ActivationFunctionType.Exp)
                                if p == 0:
                                    nc.vector.tensor_copy(s_den[:], ee[:])
                                else:
                                    nc.vector.tensor_add(s_den[:], s_den[:],
                                                         ee[:])
                                # O (+)= ee_bcast * v
                                O_v = O[:].rearrange("p (h d) -> p h d", d=D)
                                v_v = vb[:].rearrange("p (h d) -> p h d", d=D)
                                e_b = _bcast(ee[:], D)
                                if p == 0:
                                    nc.vector.tensor_tensor(
                                        O_v, v_v, e_b, mybir.AluOpType.mult)
                                else:
                                    tmp = pbsc.tile([128, CH], F32, tag="prod")
                                    tmp_v = tmp[:].rearrange(
                                        "p (h d) -> p h d", d=D)
                                    nc.vector.tensor_tensor(
                                        tmp_v, v_v, e_b, mybir.AluOpType.mult)
                                    nc.vector.tensor_add(O[:], O[:], tmp[:])
                            # normalize into the SBUF-resident ob half
                            s_inv = pbsm.tile([128, HPH], F32, tag="sinv")
                            nc.vector.reciprocal(s_inv[:], s_den[:])
                            nc.vector.tensor_tensor(
                                ob[j][:, o0:o0 + CH].rearrange(
                                    "p (h d) -> p h d", d=D),
                                O[:].rearrange("p (h d) -> p h d", d=D),
                                _bcast(s_inv[:], D), mybir.AluOpType.mult)

                # ---- Phase C0: PE-transpose ob -> oT (overlaps Wo DMA) ----
                with tc.tile_pool(name="pcoT", bufs=1) as pcoT:
                    oT = []
                    with tc.tile_pool(name="pctp", bufs=4, space="PSUM") as pctp:
                        for j in range(nt):
                            oT_j = pcoT.tile([128, CT * 128], BF16,
                                             tag=f"oT{j}")
                            oT.append(oT_j)
                            for t in range(CT):
                                tp = pctp.tile([128, 128], BF16, tag="tp")
                                nc.tensor.transpose(
                                    tp[:], ob[j][:, t * 128:(t + 1) * 128],
                                    ident[:])
                                dst = oT_j[:, t * 128:(t + 1) * 128]
                                if t % 2 == 0:
                                    nc.scalar.copy(dst, tp[:])
                                else:
                                    nc.vector.tensor_copy(dst, tp[:])

                    # ---- Phase C: result = (oT @ Wo.T) * g ----
                    with tc.tile_pool(name="pcw", bufs=1) as pcw, \
                         tc.tile_pool(name="pcg", bufs=2) as pcg, \
                         tc.tile_pool(name="pcf", bufs=2) as pcf, \
                         tc.tile_pool(name="pcps", bufs=2, space="PSUM") as pcps:
                        wo_v = load_w(pcw, wo, "wo")
                        for j in range(nt):
                            gb = pcg.tile([128, C], BF16, tag="g")
                            nc.sync.dma_start(out=gb[:], in_=g_spill[j])
                            oT_v = oT[j][:].rearrange("p (t n) -> p t n", t=CT)
                            fp = pcps.tile([128, C], F32, tag="fp")
                            for t in range(CT):
                                for c4 in range(C // 512):
                                    sl = slice(c4 * 512, (c4 + 1) * 512)
                                    nc.tensor.matmul(
                                        fp[:, sl], oT_v[:, t, :],
                                        wo_v[:, t, sl],
                                        start=(t == 0), stop=(t == CT - 1))
                            fb = pcf.tile([128, C], F32, tag="fb")
                            nc.vector.tensor_mul(fb[:], fp[:], gb[:])
                            nc.sync.dma_start(
                                out=out[j * 128:(j + 1) * 128, :], in_=fb[:])

    nc.compile()
    return nc


_NC_CACHE = {}


def _get_nc(nt=NT):
    if nt not in _NC_CACHE:
        _NC_CACHE[nt] = build_nc(nt)
    return _NC_CACHE[nt]


def prep_core_inputs(x, prefix, Wq, Wk, Wv, Wo, Wg, bg):
    """Shard + lay out host inputs for the 8 cores."""
    wqt = np.ascontiguousarray(Wq.T * SCALE).astype(NPBF16)
    wkt = np.ascontiguousarray(Wk.T).astype(NPBF16)
    wvt = np.ascontiguousarray(Wv.T).astype(NPBF16)
    wot = np.ascontiguousarray(Wo.T).astype(NPBF16)
    wgt = np.ascontiguousarray(Wg.T).astype(NPBF16)
    bgb = np.ascontiguousarray(bg.reshape(1, C)).astype(NPBF16)
    bgrb = np.ascontiguousarray(np.broadcast_to(bgb, (128, C)))
    in_maps = []
    for c in range(NCORES):
        sl = slice(c * NTOK, (c + 1) * NTOK)
        xT = np.ascontiguousarray(x[sl].T).astype(NPBF16)           # [C, NTOK]
        pT = np.ascontiguousarray(prefix[sl].transpose(2, 1, 0)).astype(NPBF16)
        in_maps.append({"xT": xT, "pT": pT, "wq": wqt, "wk": wkt,
                        "wv": wvt, "wo": wot, "wg": wgt, "bg": bgb,
                        "bgr": bgrb})
    return in_maps


def kernel(x, prefix, Wq, Wk, Wv, Wo, Wg, bg):
    from concourse.bass_utils import run_bass_kernel_spmd
    x = np.asarray(x, dtype=np.float32)
    prefix = np.asarray(prefix, dtype=np.float32)
    in_maps = prep_core_inputs(x, prefix, np.asarray(Wq), np.asarray(Wk),
                               np.asarray(Wv), np.asarray(Wo), np.asarray(Wg),
                               np.asarray(bg))
    nc = _get_nc()
    res = run_bass_kernel_spmd(nc, in_maps, core_ids=list(range(NCORES)))
    return np.concatenate([res.results[c]["out"] for c in range(NCORES)], axis=0)


# revision 26
# speedup vs baseline: 1.0684x; 1.0684x over previous
"""Trainium2 Bass kernel for LinearTransformerExpert.

Reference computation (per token n, 16 heads, head_dim 128, prefix len 8):
    q = x @ Wq.T ;  k = prefix @ Wk.T ;  v = prefix @ Wv.T
    scores[n,h,p] = q[n,h,:] . k[n,p,h,:] / sqrt(D)
    attn = softmax_p(scores);  out[n,h,:] = sum_p attn * v
    result = (out @ Wo.T) * sigmoid(x @ Wg.T + bg)

Strategy: data-parallel over the 8192 tokens across 8 NeuronCores (1024
tokens each), no collectives. All matmuls run in bf16 on the PE with the
activations as the stationary operand, so every projection lands in PSUM
token-major ([token, out_channel]) — exactly the layout the attention
(computed on DVE/ACT, hidden under the PE) wants.

v2.1 layout: phases split into half-output-channel passes so each pass
needs only 8MB of weights before its first matmul, and the next pass's
weights stream in under compute.
  A (x2 passes): q half (+gate z half, sigmoid) from x; q/g spilled to
     DRAM bf16 (cheap, overlapped). x tiles stay resident across passes.
  B (x2 passes): k/v half projections + attention on those 8 heads, fused
     per (tile, p); attention output ob written by halves, stays in SBUF.
  C0: PE-transpose ob -> oT (SBUF), overlapping the Wo weight DMA.
  C: final = (oT @ Wo.T) * g, streamed out.
Wq is pre-scaled by 1/sqrt(D) on host.
"""

import math

import numpy as np
import ml_dtypes

import concourse.bass as bass
import concourse.bacc as bacc
import concourse.mybir as mybir
from concourse import tile
from concourse.masks import make_identity

BF16 = mybir.dt.bfloat16
F32 = mybir.dt.float32
FP8 = mybir.dt.float8e4
NPBF16 = ml_dtypes.bfloat16
NPFP8 = mybir.dt.np(mybir.dt.float8e4)

# problem shape (hardcoded; kernel.py must be self-contained)
N, P, C, H = 8192, 8, 2048, 16
D = C // H                    # 128
NCORES = 8
NTOK = N // NCORES            # 1024 tokens per core
NT = NTOK // 128              # 8 token-tiles per core
CT = C // 128                 # 16 contraction tiles
OH = 2                        # output-channel halves (1024 each)
CH = C // OH                  # 1024
HPH = H // OH                 # heads per half = 8
SCALE = 1.0 / math.sqrt(D)
# k/v contraction: first CT8 c-tiles in bf16, last F8T c-tile-pairs in
# fp8e4 DoubleRow (error budget: ~4.3% * sqrt(2/16) ~= 1.5% L2, gate 2e-2).
F8T = 1                       # fp8 c-tile PAIRS (2 c-tiles)
CT8 = CT - 2 * F8T            # bf16 c-tiles for k/v = 14
WKVS = 1024.0                 # k/v weight prescale (2^10, exact in bf16/fp8)


def _bcast(ap, n):
    """Append a 0-stride broadcast dim of size n to an AP."""
    return bass.AP(ap.tensor, ap.offset, list(ap.ap) + [[0, n]])


def build_nc(nt=NT):
    """Build the per-core SPMD program for `nt` token-tiles (nt*128 tokens)."""
    ntok = nt * 128
    nc = bacc.Bacc("TRN2", target_bir_lowering=False, debug=False,
                   num_devices=NCORES)

    xT = nc.dram_tensor("xT", [C, ntok], BF16, kind="ExternalInput")
    pT = nc.dram_tensor("pT", [C, P, ntok], BF16, kind="ExternalInput")
    # fp8 copies of the last 2*F8T prefix c-tiles, paired for DoubleRow
    pT8 = nc.dram_tensor("pT8", [128, 2 * F8T, P, ntok], FP8,
                         kind="ExternalInput")
    wk8 = nc.dram_tensor("wk8", [128, 2 * F8T, C], FP8, kind="ExternalInput")
    wv8 = nc.dram_tensor("wv8", [128, 2 * F8T, C], FP8, kind="ExternalInput")
    wq = nc.dram_tensor("wq", [C, C], BF16, kind="ExternalInput")
    wk = nc.dram_tensor("wk", [C, C], BF16, kind="ExternalInput")
    wv = nc.dram_tensor("wv", [C, C], BF16, kind="ExternalInput")
    wo = nc.dram_tensor("wo", [C, C], BF16, kind="ExternalInput")
    wg = nc.dram_tensor("wg", [C, C], BF16, kind="ExternalInput")
    bg = nc.dram_tensor("bg", [1, C], BF16, kind="ExternalInput")
    bgr = nc.dram_tensor("bgr", [128, C], BF16, kind="ExternalInput")
    out = nc.dram_tensor("out", [ntok, C], F32, kind="ExternalOutput")

    q_spill = nc.dram_tensor("q_spill", [nt, 128, C], BF16)
    g_spill = nc.dram_tensor("g_spill", [nt, 128, C], BF16)

    def wview(w):  # [C, C] dram -> [128, CT, C] (partition, ktile, outch)
        return w[:].rearrange("(t p) o -> p t o", p=128)

    def load_w(pool, w, tag, o0=0, owid=C, tn=CT):
        """Chunked (per c-tile) weight DMA into an SBUF tile; returns 3D view."""
        sb = pool.tile([128, tn * owid], BF16, tag=tag)
        v = sb[:].rearrange("p (t o) -> p t o", t=tn)
        wv_ = wview(w)
        for t in range(tn):
            nc.sync.dma_start(out=v[:, t, :], in_=wv_[:, t, o0:o0 + owid])
        return v

    def load_w8(pool, w8, tag, o0, owid):
        """fp8 DoubleRow weight pair [128, 2*F8T, owid] for one half."""
        sb = pool.tile([128, 2 * F8T * owid], FP8, tag=tag)
        v = sb[:].rearrange("p (i o) -> p i o", i=2 * F8T)
        nc.sync.dma_start(out=v, in_=w8[:, :, o0:o0 + owid])
        return v

    def xview(j):  # stationary block for token-tile j: [128, CT, 128]
        return xT[:, j * 128:(j + 1) * 128].rearrange("(t p) n -> p t n", p=128)

    def pview(j, p):
        return pT[:, p, j * 128:(j + 1) * 128].rearrange("(t p) n -> p t n", p=128)

    with tile.TileContext(nc) as tc:
        with tc.tile_pool(name="const", bufs=1) as pconst:
            ident = pconst.tile([128, 128], BF16, tag="ident")
            make_identity(nc, ident[:])

            # ---------- Phase A: q = x@Wq.T (pre-scaled), g = sigmoid(x@Wg.T+bg)
            # Two half-output passes; x tiles resident across both.
            with tc.tile_pool(name="pax", bufs=1) as pax:
                xb_v = []
                with tc.tile_pool(name="paw", bufs=2) as paw, \
                     tc.tile_pool(name="pao", bufs=3) as pao, \
                     tc.tile_pool(name="papq", bufs=2, space="PSUM") as papq, \
                     tc.tile_pool(name="papz", bufs=2, space="PSUM") as papz:
                    bgr_sb = pax.tile([128, C], BF16, tag="bgr")
                    nc.sync.dma_start(out=bgr_sb[:], in_=bgr[:])
                    for hf in range(OH):
                        o0 = hf * CH
                        wq_v = load_w(paw, wq, "wq", o0, CH)
                        wg_v = load_w(paw, wg, "wg", o0, CH)
                        for j in range(nt):
                            if hf == 0:
                                xb = pax.tile([128, CT * 128], BF16,
                                              tag=f"xb{j}")
                                xb_v.append(xb[:].rearrange(
                                    "p (t n) -> p t n", t=CT))
                                nc.sync.dma_start(out=xb_v[j], in_=xview(j))
                            qp = papq.tile([128, CH], F32, tag="qp")
                            zp = papz.tile([128, CH], F32, tag="zp")
                            for t in range(CT):
                                for c2 in range(CH // 512):
                                    sl = slice(c2 * 512, (c2 + 1) * 512)
                                    nc.tensor.matmul(
                                        qp[:, sl], xb_v[j][:, t, :],
                                        wq_v[:, t, sl],
                                        start=(t == 0), stop=(t == CT - 1))
                                    nc.tensor.matmul(
                                        zp[:, sl], xb_v[j][:, t, :],
                                        wg_v[:, t, sl],
                                        start=(t == 0), stop=(t == CT - 1))
                            qb = pao.tile([128, CH], BF16, tag="qb")
                            nc.scalar.copy(qb[:], qp[:])
                            nc.sync.dma_start(
                                out=q_spill[j, :, o0:o0 + CH], in_=qb[:])
                            zb = pao.tile([128, CH], BF16, tag="zb")
                            nc.vector.tensor_add(
                                zb[:], zp[:], bgr_sb[:, o0:o0 + CH])
                            gb = pao.tile([128, CH], BF16, tag="gb")
                            nc.scalar.activation(
                                gb[:], zb[:],
                                mybir.ActivationFunctionType.Sigmoid)
                            nc.sync.dma_start(
                                out=g_spill[j, :, o0:o0 + CH], in_=gb[:])

            # ---------- Phase B: k/v projection + attention, fused ----------
            # Two half-output passes (8 heads each); ob written by halves.
            with tc.tile_pool(name="pbob", bufs=1) as pbob:
                ob = []
                with tc.tile_pool(name="pbw", bufs=2) as pbw, \
                     tc.tile_pool(name="pbpf", bufs=3) as pbpf, \
                     tc.tile_pool(name="pbq", bufs=2) as pbq, \
                     tc.tile_pool(name="pbkv", bufs=2) as pbkv, \
                     tc.tile_pool(name="pbO", bufs=2) as pbO, \
                     tc.tile_pool(name="pbsc", bufs=2) as pbsc, \
                     tc.tile_pool(name="pbsm", bufs=3) as pbsm, \
                     tc.tile_pool(name="pbpsk", bufs=2, space="PSUM") as pbpsk, \
                     tc.tile_pool(name="pbpsv", bufs=2, space="PSUM") as pbpsv:
                    for hf in range(OH):
                        o0 = hf * CH
                        wk_v = load_w(pbw, wk, "wk", o0, CH, tn=CT8)
                        wv_v = load_w(pbw, wv, "wv", o0, CH, tn=CT8)
                        wk8_v = load_w8(pbw, wk8, "wk8", o0, CH)
                        wv8_v = load_w8(pbw, wv8, "wv8", o0, CH)
                        for j in range(nt):
                            qb = pbq.tile([128, CH], BF16, tag="q")
                            nc.sync.dma_start(out=qb[:],
                                              in_=q_spill[j, :, o0:o0 + CH])
                            if hf == 0:
                                ob_j = pbob.tile([128, C], BF16, tag=f"ob{j}")
                                ob.append(ob_j)
                            O = pbO.tile([128, CH], F32, tag="O")
                            s_den = pbsm.tile([128, HPH], F32, tag="sden")
                            for p in range(P):
                                pf = pbpf.tile([128, CT8 * 128], BF16,
                                               tag="pf")
                                pf_v = pf[:].rearrange("p (t n) -> p t n",
                                                       t=CT8)
                                nc.sync.dma_start(out=pf_v,
                                                  in_=pview(j, p)[:, :CT8, :])
                                pf8 = pbpf.tile([128, 2 * F8T * 128], FP8,
                                                tag="pf8")
                                pf8_v = pf8[:].rearrange("p (i n) -> p i n",
                                                         i=2 * F8T)
                                nc.sync.dma_start(
                                    out=pf8_v,
                                    in_=pT8[:, :, p, j * 128:(j + 1) * 128])
                                kp = pbpsk.tile([128, CH], F32, tag="kp")
                                vp = pbpsv.tile([128, CH], F32, tag="vp")
                                for t in range(CT8):
                                    for c2 in range(CH // 512):
                                        sl = slice(c2 * 512, (c2 + 1) * 512)
                                        nc.tensor.matmul(
                                            kp[:, sl], pf_v[:, t, :],
                                            wk_v[:, t, sl],
                                            start=(t == 0), stop=False)
                                        nc.tensor.matmul(
                                            vp[:, sl], pf_v[:, t, :],
                                            wv_v[:, t, sl],
                                            start=(t == 0), stop=False)
                                # last 2 c-tiles: one fp8 DoubleRow MM each
                                for c2 in range(CH // 512):
                                    sl = slice(c2 * 512, (c2 + 1) * 512)
                                    nc.tensor.matmul(
                                        kp[:, sl], pf8_v, wk8_v[:, :, sl],
                                        start=False, stop=True,
                                        perf_mode=mybir.MatmulPerfMode.DoubleRow)
                                    nc.tensor.matmul(
                                        vp[:, sl], pf8_v, wv8_v[:, :, sl],
                                        start=False, stop=True,
                                        perf_mode=mybir.MatmulPerfMode.DoubleRow)
                                kb = pbkv.tile([128, CH], BF16, tag="kb")
                                vb = pbkv.tile([128, CH], BF16, tag="vb")
                                nc.scalar.mul(kb[:], kp[:], 1.0 / WKVS)
                                nc.vector.tensor_scalar_mul(vb[:], vp[:],
                                                            1.0 / WKVS)
                                # scores for the 8 heads of this half
                                prod = pbsc.tile([128, CH], F32, tag="prod")
                                nc.vector.tensor_mul(prod[:], qb[:], kb[:])
                                sc = pbsm.tile([128, HPH], F32, tag="sc")
                                nc.vector.tensor_reduce(
                                    sc[:],
                                    prod[:].rearrange("p (h d) -> p h d", d=D),
                                    mybir.AxisListType.X, mybir.AluOpType.add)
                                ee = pbsm.tile([128, HPH], F32, tag="ee")
                                nc.scalar.activation(
                                    ee[:], sc[:],
                                    mybir.ActivationFunctionType.Exp)
                                if p == 0:
                                    nc.vector.tensor_copy(s_den[:], ee[:])
                                else:
                                    nc.vector.tensor_add(s_den[:], s_den[:],
                                                         ee[:])
                                # O (+)= ee_bcast * v
                                O_v = O[:].rearrange("p (h d) -> p h d", d=D)
                                v_v = vb[:].rearrange("p (h d) -> p h d", d=D)
                                e_b = _bcast(ee[:], D)
                                if p == 0:
                                    nc.vector.tensor_tensor(
                                        O_v, v_v, e_b, mybir.AluOpType.mult)
                                else:
                                    tmp = pbsc.tile([128, CH], F32, tag="prod")
                                    tmp_v = tmp[:].rearrange(
                                        "p (h d) -> p h d", d=D)
                                    nc.vector.tensor_tensor(
                                        tmp_v, v_v, e_b, mybir.AluOpType.mult)
                                    nc.vector.tensor_add(O[:], O[:], tmp[:])
                            # normalize into the SBUF-resident ob half
                            s_inv = pbsm.tile([128, HPH], F32, tag="sinv")
                            nc.vector.reciprocal(s_inv[:], s_den[:])
                            nc.vector.tensor_tensor(
                                ob[j][:, o0:o0 + CH].rearrange(
                                    "p (h d) -> p h d", d=D),
                                O[:].rearrange("p (h d) -> p h d", d=D),
                                _bcast(s_inv[:], D), mybir.AluOpType.mult)

                # ---- Phase C0: PE-transpose ob -> oT (overlaps Wo DMA) ----
                with tc.tile_pool(name="pcoT", bufs=1) as pcoT:
                    oT = []
                    with tc.tile_pool(name="pctp", bufs=4, space="PSUM") as pctp:
                        for j in range(nt):
                            oT_j = pcoT.tile([128, CT * 128], BF16,
                                             tag=f"oT{j}")
                            oT.append(oT_j)
                            for t in range(CT):
                                tp = pctp.tile([128, 128], BF16, tag="tp")
                                nc.tensor.transpose(
                                    tp[:], ob[j][:, t * 128:(t + 1) * 128],
                                    ident[:])
                                dst = oT_j[:, t * 128:(t + 1) * 128]
                                if t % 2 == 0:
                                    nc.scalar.copy(dst, tp[:])
                                else:
                                    nc.vector.tensor_copy(dst, tp[:])

                    # ---- Phase C: result = (oT @ Wo.T) * g ----
                    with tc.tile_pool(name="pcw", bufs=1) as pcw, \
                         tc.tile_pool(name="pcg", bufs=2) as pcg, \
                         tc.tile_pool(name="pcf", bufs=2) as pcf, \
                         tc.tile_pool(name="pcps", bufs=2, space="PSUM") as pcps:
                        wo_v = load_w(pcw, wo, "wo")
                        for j in range(nt):
                            gb = pcg.tile([128, C], BF16, tag="g")
                            nc.sync.dma_start(out=gb[:], in_=g_spill[j])
                            oT_v = oT[j][:].rearrange("p (t n) -> p t n", t=CT)
                            fp = pcps.tile([128, C], F32, tag="fp")
                            for t in range(CT):
                                for c4 in range(C // 512):
                                    sl = slice(c4 * 512, (c4 + 1) * 512)
                                    nc.tensor.matmul(
                                        fp[:, sl], oT_v[:, t, :],
                                        wo_v[:, t, sl],
                                        start=(t == 0), stop=(t == CT - 1))
                            fb = pcf.tile([128, C], F32, tag="fb")
                            nc.vector.tensor_mul(fb[:], fp[:], gb[:])
                            nc.sync.dma_start(
                                out=out[j * 128:(j + 1) * 128, :], in_=fb[:])

    nc.compile()
    return nc


_NC_CACHE = {}


def _get_nc(nt=NT):
    if nt not in _NC_CACHE:
        _NC_CACHE[nt] = build_nc(nt)
    return _NC_CACHE[nt]


def prep_core_inputs(x, prefix, Wq, Wk, Wv, Wo, Wg, bg):
    """Shard + lay out host inputs for the 8 cores."""
    wqt = np.ascontiguousarray(Wq.T * SCALE).astype(NPBF16)
    # k/v weights prescaled by 2^10 so the bf16 and fp8 partial sums share
    # one PSUM scale; the drain copies divide it back out.
    wkt = np.ascontiguousarray(Wk.T * WKVS).astype(NPBF16)
    wvt = np.ascontiguousarray(Wv.T * WKVS).astype(NPBF16)
    wot = np.ascontiguousarray(Wo.T).astype(NPBF16)
    wgt = np.ascontiguousarray(Wg.T).astype(NPBF16)

    def w8pair(W):  # fp8 pair layout [128, 2*F8T, C] of the last c-tiles
        rows = (W.T * WKVS)[CT8 * 128:, :]              # [2*F8T*128, C]
        return np.ascontiguousarray(
            rows.reshape(2 * F8T, 128, C).transpose(1, 0, 2)).astype(NPFP8)

    wk8p = w8pair(Wk)
    wv8p = w8pair(Wv)
    bgb = np.ascontiguousarray(bg.reshape(1, C)).astype(NPBF16)
    bgrb = np.ascontiguousarray(np.broadcast_to(bgb, (128, C)))
    in_maps = []
    for c in range(NCORES):
        sl = slice(c * NTOK, (c + 1) * NTOK)
        xT = np.ascontiguousarray(x[sl].T).astype(NPBF16)           # [C, NTOK]
        pTc = prefix[sl].transpose(2, 1, 0)                         # [C, P, NTOK]
        pT = np.ascontiguousarray(pTc).astype(NPBF16)
        pT8 = np.ascontiguousarray(
            pTc[CT8 * 128:].reshape(2 * F8T, 128, P, NTOK)
            .transpose(1, 0, 2, 3)).astype(NPFP8)
        in_maps.append({"xT": xT, "pT": pT, "pT8": pT8, "wq": wqt, "wk": wkt,
                        "wv": wvt, "wk8": wk8p, "wv8": wv8p, "wo": wot,
                        "wg": wgt, "bg": bgb, "bgr": bgrb})
    return in_maps


def kernel(x, prefix, Wq, Wk, Wv, Wo, Wg, bg):
    from concourse.bass_utils import run_bass_kernel_spmd
    x = np.asarray(x, dtype=np.float32)
    prefix = np.asarray(prefix, dtype=np.float32)
    in_maps = prep_core_inputs(x, prefix, np.asarray(Wq), np.asarray(Wk),
                               np.asarray(Wv), np.asarray(Wo), np.asarray(Wg),
                               np.asarray(bg))
    nc = _get_nc()
    res = run_bass_kernel_spmd(nc, in_maps, core_ids=list(range(NCORES)))
    return np.concatenate([res.results[c]["out"] for c in range(NCORES)], axis=0)


# revision 31
# speedup vs baseline: 1.0814x; 1.0121x over previous
"""Trainium2 Bass kernel for LinearTransformerExpert.

Reference computation (per token n, 16 heads, head_dim 128, prefix len 8):
    q = x @ Wq.T ;  k = prefix @ Wk.T ;  v = prefix @ Wv.T
    scores[n,h,p] = q[n,h,:] . k[n,p,h,:] / sqrt(D)
    attn = softmax_p(scores);  out[n,h,:] = sum_p attn * v
    result = (out @ Wo.T) * sigmoid(x @ Wg.T + bg)

Strategy: data-parallel over the 8192 tokens across 8 NeuronCores (1024
tokens each), no collectives. All matmuls run in bf16 on the PE with the
activations as the stationary operand, so every projection lands in PSUM
token-major ([token, out_channel]) — exactly the layout the attention
(computed on DVE/ACT, hidden under the PE) wants.

v2.1 layout: phases split into half-output-channel passes so each pass
needs only 8MB of weights before its first matmul, and the next pass's
weights stream in under compute.
  A (x2 passes): q half (+gate z half, sigmoid) from x; q/g spilled to
     DRAM bf16 (cheap, overlapped). x tiles stay resident across passes.
  B (x2 passes): k/v half projections + attention on those 8 heads, fused
     per (tile, p); attention output ob written by halves, stays in SBUF.
  C0: PE-transpose ob -> oT (SBUF), overlapping the Wo weight DMA.
  C: final = (oT @ Wo.T) * g, streamed out.
Wq is pre-scaled by 1/sqrt(D) on host.
"""

import math

import numpy as np
import ml_dtypes

import concourse.bass as bass
import concourse.bacc as bacc
import concourse.mybir as mybir
from concourse import tile
from concourse.masks import make_identity

BF16 = mybir.dt.bfloat16
F32 = mybir.dt.float32
FP8 = mybir.dt.float8e4
NPBF16 = ml_dtypes.bfloat16
NPFP8 = mybir.dt.np(mybir.dt.float8e4)

# problem shape (hardcoded; kernel.py must be self-contained)
N, P, C, H = 8192, 8, 2048, 16
D = C // H                    # 128
NCORES = 8
NTOK = N // NCORES            # 1024 tokens per core
NT = NTOK // 128              # 8 token-tiles per core
CT = C // 128                 # 16 contraction tiles
OH = 2                        # output-channel halves (1024 each)
CH = C // OH                  # 1024
HPH = H // OH                 # heads per half = 8
SCALE = 1.0 / math.sqrt(D)
# k/v contraction: first CT8 c-tiles in bf16, last F8T c-tile-pairs in
# fp8e4 DoubleRow (error budget: ~4.3% * sqrt(2/16) ~= 1.5% L2, gate 2e-2).
F8T = 1                       # fp8 c-tile PAIRS (2 c-tiles)
CT8 = CT - 2 * F8T            # bf16 c-tiles for k/v = 14
WKVS = 1024.0                 # k/v weight prescale (2^10, exact in bf16/fp8)


def _bcast(ap, n):
    """Append a 0-stride broadcast dim of size n to an AP."""
    return bass.AP(ap.tensor, ap.offset, list(ap.ap) + [[0, n]])


def build_nc(nt=NT):
    """Build the per-core SPMD program for `nt` token-tiles (nt*128 tokens)."""
    ntok = nt * 128
    nc = bacc.Bacc("TRN2", target_bir_lowering=False, debug=False,
                   num_devices=NCORES)

    xT = nc.dram_tensor("xT", [C, ntok], BF16, kind="ExternalInput")
    pT = nc.dram_tensor("pT", [C, P, ntok], BF16, kind="ExternalInput")
    # fp8 copies of the last 2*F8T prefix c-tiles, paired for DoubleRow
    pT8 = nc.dram_tensor("pT8", [128, 2 * F8T, P, ntok], FP8,
                         kind="ExternalInput")
    wk8 = nc.dram_tensor("wk8", [128, 2 * F8T, C], FP8, kind="ExternalInput")
    wv8 = nc.dram_tensor("wv8", [128, 2 * F8T, C], FP8, kind="ExternalInput")
    xT8 = nc.dram_tensor("xT8", [128, 2 * F8T, ntok], FP8,
                         kind="ExternalInput")
    wg8 = nc.dram_tensor("wg8", [128, 2 * F8T, C], FP8, kind="ExternalInput")
    wq = nc.dram_tensor("wq", [C, C], BF16, kind="ExternalInput")
    wk = nc.dram_tensor("wk", [C, C], BF16, kind="ExternalInput")
    wv = nc.dram_tensor("wv", [C, C], BF16, kind="ExternalInput")
    wo = nc.dram_tensor("wo", [C, C], BF16, kind="ExternalInput")
    wg = nc.dram_tensor("wg", [C, C], BF16, kind="ExternalInput")
    bg = nc.dram_tensor("bg", [1, C], BF16, kind="ExternalInput")
    bgr = nc.dram_tensor("bgr", [128, C], BF16, kind="ExternalInput")
    out = nc.dram_tensor("out", [ntok, C], F32, kind="ExternalOutput")

    q_spill = nc.dram_tensor("q_spill", [nt, 128, C], BF16)
    g_spill = nc.dram_tensor("g_spill", [nt, 128, C], BF16)

    def wview(w):  # [C, C] dram -> [128, CT, C] (partition, ktile, outch)
        return w[:].rearrange("(t p) o -> p t o", p=128)

    def load_w(pool, w, tag, o0=0, owid=C, tn=CT):
        """Chunked (per c-tile) weight DMA into an SBUF tile; returns 3D view."""
        sb = pool.tile([128, tn * owid], BF16, tag=tag)
        v = sb[:].rearrange("p (t o) -> p t o", t=tn)
        wv_ = wview(w)
        for t in range(tn):
            nc.sync.dma_start(out=v[:, t, :], in_=wv_[:, t, o0:o0 + owid])
        return v

    def load_w8(pool, w8, tag, o0, owid):
        """fp8 DoubleRow weight pair [128, 2*F8T, owid] for one half."""
        sb = pool.tile([128, 2 * F8T * owid], FP8, tag=tag)
        v = sb[:].rearrange("p (i o) -> p i o", i=2 * F8T)
        nc.sync.dma_start(out=v, in_=w8[:, :, o0:o0 + owid])
        return v

    def xview(j):  # stationary block for token-tile j: [128, CT, 128]
        return xT[:, j * 128:(j + 1) * 128].rearrange("(t p) n -> p t n", p=128)

    def pview(j, p):
        return pT[:, p, j * 128:(j + 1) * 128].rearrange("(t p) n -> p t n", p=128)

    with tile.TileContext(nc) as tc:
        with tc.tile_pool(name="const", bufs=1) as pconst:
            ident = pconst.tile([128, 128], BF16, tag="ident")
            make_identity(nc, ident[:])

            # ---------- Phase A: q = x@Wq.T (pre-scaled), g = sigmoid(x@Wg.T+bg)
            # Two half-output passes; x tiles resident across both.
            with tc.tile_pool(name="pax", bufs=1) as pax:
                xb_v = []
                with tc.tile_pool(name="paw", bufs=2) as paw, \
                     tc.tile_pool(name="pao", bufs=3) as pao, \
                     tc.tile_pool(name="papq", bufs=2, space="PSUM") as papq, \
                     tc.tile_pool(name="papz", bufs=2, space="PSUM") as papz:
                    bgr_sb = pax.tile([128, C], BF16, tag="bgr")
                    nc.sync.dma_start(out=bgr_sb[:], in_=bgr[:])
                    x8_v = []
                    for hf in range(OH):
                        o0 = hf * CH
                        wq_v = load_w(paw, wq, "wq", o0, CH)
                        wg_v = load_w(paw, wg, "wg", o0, CH, tn=CT8)
                        wg8_v = load_w8(paw, wg8, "wg8", o0, CH)
                        for j in range(nt):
                            if hf == 0:
                                xb = pax.tile([128, CT * 128], BF16,
                                              tag=f"xb{j}")
                                xb_v.append(xb[:].rearrange(
                                    "p (t n) -> p t n", t=CT))
                                nc.sync.dma_start(out=xb_v[j], in_=xview(j))
                                x8 = pax.tile([128, 2 * F8T * 128], FP8,
                                              tag=f"x8{j}")
                                x8_v.append(x8[:].rearrange(
                                    "p (i n) -> p i n", i=2 * F8T))
                                nc.sync.dma_start(
                                    out=x8_v[j],
                                    in_=xT8[:, :, j * 128:(j + 1) * 128])
                            qp = papq.tile([128, CH], F32, tag="qp")
                            zp = papz.tile([128, CH], F32, tag="zp")
                            for t in range(CT):
                                for c2 in range(CH // 512):
                                    sl = slice(c2 * 512, (c2 + 1) * 512)
                                    nc.tensor.matmul(
                                        qp[:, sl], xb_v[j][:, t, :],
                                        wq_v[:, t, sl],
                                        start=(t == 0), stop=(t == CT - 1))
                                    if t < CT8:
                                        nc.tensor.matmul(
                                            zp[:, sl], xb_v[j][:, t, :],
                                            wg_v[:, t, sl],
                                            start=(t == 0), stop=False)
                            for c2 in range(CH // 512):
                                sl = slice(c2 * 512, (c2 + 1) * 512)
                                nc.tensor.matmul(
                                    zp[:, sl], x8_v[j], wg8_v[:, :, sl],
                                    start=False, stop=True,
                                    perf_mode=mybir.MatmulPerfMode.DoubleRow)
                            qb = pao.tile([128, CH], BF16, tag="qb")
                            nc.scalar.copy(qb[:], qp[:])
                            nc.sync.dma_start(
                                out=q_spill[j, :, o0:o0 + CH], in_=qb[:])
                            zb = pao.tile([128, CH], BF16, tag="zb")
                            nc.vector.scalar_tensor_tensor(
                                zb[:], zp[:], 1.0 / WKVS,
                                bgr_sb[:, o0:o0 + CH],
                                mybir.AluOpType.mult, mybir.AluOpType.add)
                            gb = pao.tile([128, CH], BF16, tag="gb")
                            nc.scalar.activation(
                                gb[:], zb[:],
                                mybir.ActivationFunctionType.Sigmoid)
                            nc.sync.dma_start(
                                out=g_spill[j, :, o0:o0 + CH], in_=gb[:])

            # ---------- Phase B: k/v projection + attention, fused ----------
            # Two half-output passes (8 heads each); ob written by halves.
            with tc.tile_pool(name="pbob", bufs=1) as pbob:
                ob = []
                with tc.tile_pool(name="pbw", bufs=2) as pbw, \
                     tc.tile_pool(name="pbpf", bufs=3) as pbpf, \
                     tc.tile_pool(name="pbq", bufs=2) as pbq, \
                     tc.tile_pool(name="pbkv", bufs=2) as pbkv, \
                     tc.tile_pool(name="pbO", bufs=2) as pbO, \
                     tc.tile_pool(name="pbsc", bufs=2) as pbsc, \
                     tc.tile_pool(name="pbsm", bufs=3) as pbsm, \
                     tc.tile_pool(name="pbpsk", bufs=2, space="PSUM") as pbpsk, \
                     tc.tile_pool(name="pbpsv", bufs=2, space="PSUM") as pbpsv:
                    for hf in range(OH):
                        o0 = hf * CH
                        wk_v = load_w(pbw, wk, "wk", o0, CH, tn=CT8)
                        wv_v = load_w(pbw, wv, "wv", o0, CH, tn=CT8)
                        wk8_v = load_w8(pbw, wk8, "wk8", o0, CH)
                        wv8_v = load_w8(pbw, wv8, "wv8", o0, CH)
                        for j in range(nt):
                            qb = pbq.tile([128, CH], BF16, tag="q")
                            nc.sync.dma_start(out=qb[:],
                                              in_=q_spill[j, :, o0:o0 + CH])
                            if hf == 0:
                                ob_j = pbob.tile([128, C], BF16, tag=f"ob{j}")
                                ob.append(ob_j)
                            O = pbO.tile([128, CH], F32, tag="O")
                            s_den = pbsm.tile([128, HPH], F32, tag="sden")
                            for p in range(P):
                                pf = pbpf.tile([128, CT8 * 128], BF16,
                                               tag="pf")
                                pf_v = pf[:].rearrange("p (t n) -> p t n",
                                                       t=CT8)
                                nc.sync.dma_start(out=pf_v,
                                                  in_=pview(j, p)[:, :CT8, :])
                                pf8 = pbpf.tile([128, 2 * F8T * 128], FP8,
                                                tag="pf8")
                                pf8_v = pf8[:].rearrange("p (i n) -> p i n",
                                                         i=2 * F8T)
                                nc.sync.dma_start(
                                    out=pf8_v,
                                    in_=pT8[:, :, p, j * 128:(j + 1) * 128])
                                kp = pbpsk.tile([128, CH], F32, tag="kp")
                                vp = pbpsv.tile([128, CH], F32, tag="vp")
                                for t in range(CT8):
                                    for c2 in range(CH // 512):
                                        sl = slice(c2 * 512, (c2 + 1) * 512)
                                        nc.tensor.matmul(
                                            kp[:, sl], pf_v[:, t, :],
                                            wk_v[:, t, sl],
                                            start=(t == 0), stop=False)
                                        nc.tensor.matmul(
                                            vp[:, sl], pf_v[:, t, :],
                                            wv_v[:, t, sl],
                                            start=(t == 0), stop=False)
                                # last 2 c-tiles: one fp8 DoubleRow MM each
                                for c2 in range(CH // 512):
                                    sl = slice(c2 * 512, (c2 + 1) * 512)
                                    nc.tensor.matmul(
                                        kp[:, sl], pf8_v, wk8_v[:, :, sl],
                                        start=False, stop=True,
                                        perf_mode=mybir.MatmulPerfMode.DoubleRow)
                                    nc.tensor.matmul(
                                        vp[:, sl], pf8_v, wv8_v[:, :, sl],
                                        start=False, stop=True,
                                        perf_mode=mybir.MatmulPerfMode.DoubleRow)
                                kb = pbkv.tile([128, CH], BF16, tag="kb")
                                vb = pbkv.tile([128, CH], BF16, tag="vb")
                                nc.scalar.mul(kb[:], kp[:], 1.0 / WKVS)
                                nc.vector.tensor_scalar_mul(vb[:], vp[:],
                                                            1.0 / WKVS)
                                # scores for the 8 heads of this half
                                prod = pbsc.tile([128, CH], F32, tag="prod")
                                nc.vector.tensor_mul(prod[:], qb[:], kb[:])
                                sc = pbsm.tile([128, HPH], F32, tag="sc")
                                nc.vector.tensor_reduce(
                                    sc[:],
                                    prod[:].rearrange("p (h d) -> p h d", d=D),
                                    mybir.AxisListType.X, mybir.AluOpType.add)
                                ee = pbsm.tile([128, HPH], F32, tag="ee")
                                nc.scalar.activation(
                                    ee[:], sc[:],
                                    mybir.ActivationFunctionType.Exp)
                                if p == 0:
                                    nc.vector.tensor_copy(s_den[:], ee[:])
                                else:
                                    nc.vector.tensor_add(s_den[:], s_den[:],
                                                         ee[:])
                                # O (+)= ee_bcast * v
                                O_v = O[:].rearrange("p (h d) -> p h d", d=D)
                                v_v = vb[:].rearrange("p (h d) -> p h d", d=D)
                                e_b = _bcast(ee[:], D)
                                if p == 0:
                                    nc.vector.tensor_tensor(
                                        O_v, v_v, e_b, mybir.AluOpType.mult)
                                else:
                                    tmp = pbsc.tile([128, CH], F32, tag="prod")
                                    tmp_v = tmp[:].rearrange(
                                        "p (h d) -> p h d", d=D)
                                    nc.vector.tensor_tensor(
                                        tmp_v, v_v, e_b, mybir.AluOpType.mult)
                                    nc.vector.tensor_add(O[:], O[:], tmp[:])
                            # normalize into the SBUF-resident ob half
                            s_inv = pbsm.tile([128, HPH], F32, tag="sinv")
                            nc.vector.reciprocal(s_inv[:], s_den[:])
                            nc.vector.tensor_tensor(
                                ob[j][:, o0:o0 + CH].rearrange(
                                    "p (h d) -> p h d", d=D),
                                O[:].rearrange("p (h d) -> p h d", d=D),
                                _bcast(s_inv[:], D), mybir.AluOpType.mult)

                # ---- Phase C0: PE-transpose ob -> oT (overlaps Wo DMA) ----
                with tc.tile_pool(name="pcoT", bufs=1) as pcoT:
                    oT = []
                    with tc.tile_pool(name="pctp", bufs=4, space="PSUM") as pctp:
                        for j in range(nt):
                            oT_j = pcoT.tile([128, CT * 128], BF16,
                                             tag=f"oT{j}")
                            oT.append(oT_j)
                            for t in range(CT):
                                tp = pctp.tile([128, 128], BF16, tag="tp")
                                nc.tensor.transpose(
                                    tp[:], ob[j][:, t * 128:(t + 1) * 128],
                                    ident[:])
                                dst = oT_j[:, t * 128:(t + 1) * 128]
                                if t % 2 == 0:
                                    nc.scalar.copy(dst, tp[:])
                                else:
                                    nc.vector.tensor_copy(dst, tp[:])

                    # ---- Phase C: result = (oT @ Wo.T) * g ----
                    with tc.tile_pool(name="pcw", bufs=1) as pcw, \
                         tc.tile_pool(name="pcg", bufs=2) as pcg, \
                         tc.tile_pool(name="pcf", bufs=2) as pcf, \
                         tc.tile_pool(name="pcps", bufs=2, space="PSUM") as pcps:
                        wo_v = load_w(pcw, wo, "wo")
                        for j in range(nt):
                            gb = pcg.tile([128, C], BF16, tag="g")
                            nc.sync.dma_start(out=gb[:], in_=g_spill[j])
                            oT_v = oT[j][:].rearrange("p (t n) -> p t n", t=CT)
                            fp = pcps.tile([128, C], F32, tag="fp")
                            for t in range(CT):
                                for c4 in range(C // 512):
                                    sl = slice(c4 * 512, (c4 + 1) * 512)
                                    nc.tensor.matmul(
                                        fp[:, sl], oT_v[:, t, :],
                                        wo_v[:, t, sl],
                                        start=(t == 0), stop=(t == CT - 1))
                            fb = pcf.tile([128, C], F32, tag="fb")
                            nc.vector.tensor_mul(fb[:], fp[:], gb[:])
                            nc.sync.dma_start(
                                out=out[j * 128:(j + 1) * 128, :], in_=fb[:])

    nc.compile()
    return nc


_NC_CACHE = {}


def _get_nc(nt=NT):
    if nt not in _NC_CACHE:
        _NC_CACHE[nt] = build_nc(nt)
    return _NC_CACHE[nt]


def prep_core_inputs(x, prefix, Wq, Wk, Wv, Wo, Wg, bg):
    """Shard + lay out host inputs for the 8 cores."""
    wqt = np.ascontiguousarray(Wq.T * SCALE).astype(NPBF16)
    # k/v weights prescaled by 2^10 so the bf16 and fp8 partial sums share
    # one PSUM scale; the drain copies divide it back out.
    wkt = np.ascontiguousarray(Wk.T * WKVS).astype(NPBF16)
    wvt = np.ascontiguousarray(Wv.T * WKVS).astype(NPBF16)
    wot = np.ascontiguousarray(Wo.T).astype(NPBF16)
    wgt = np.ascontiguousarray(Wg.T * WKVS).astype(NPBF16)

    def w8pair(W):  # fp8 pair layout [128, 2*F8T, C] of the last c-tiles
        rows = (W.T * WKVS)[CT8 * 128:, :]              # [2*F8T*128, C]
        return np.ascontiguousarray(
            rows.reshape(2 * F8T, 128, C).transpose(1, 0, 2)).astype(NPFP8)

    wk8p = w8pair(Wk)
    wv8p = w8pair(Wv)
    wg8p = w8pair(Wg)
    bgb = np.ascontiguousarray(bg.reshape(1, C)).astype(NPBF16)
    bgrb = np.ascontiguousarray(np.broadcast_to(bgb, (128, C)))
    in_maps = []
    for c in range(NCORES):
        sl = slice(c * NTOK, (c + 1) * NTOK)
        xT = np.ascontiguousarray(x[sl].T).astype(NPBF16)           # [C, NTOK]
        pTc = prefix[sl].transpose(2, 1, 0)                         # [C, P, NTOK]
        pT = np.ascontiguousarray(pTc).astype(NPBF16)
        pT8 = np.ascontiguousarray(
            pTc[CT8 * 128:].reshape(2 * F8T, 128, P, NTOK)
            .transpose(1, 0, 2, 3)).astype(NPFP8)
        xT8 = np.ascontiguousarray(
            xT[CT8 * 128:].astype(np.float32)
            .reshape(2 * F8T, 128, NTOK).transpose(1, 0, 2)).astype(NPFP8)
        in_maps.append({"xT": xT, "pT": pT, "pT8": pT8, "xT8": xT8,
                        "wq": wqt, "wk": wkt, "wv": wvt, "wk8": wk8p,
                        "wv8": wv8p, "wg8": wg8p, "wo": wot, "wg": wgt,
                        "bg": bgb, "bgr": bgrb})
    return in_maps


def kernel(x, prefix, Wq, Wk, Wv, Wo, Wg, bg):
    from concourse.bass_utils import run_bass_kernel_spmd
    x = np.asarray(x, dtype=np.float32)
    prefix = np.asarray(prefix, dtype=np.float32)
    in_maps = prep_core_inputs(x, prefix, np.asarray(Wq), np.asarray(Wk),
                               np.asarray(Wv), np.asarray(Wo), np.asarray(Wg),
                               np.asarray(bg))
    nc = _get_nc()
    res = run_bass_kernel_spmd(nc, in_maps, core_ids=list(range(NCORES)))
    return np.concatenate([res.results[c]["out"] for c in range(NCORES)], axis=0)
